# revision 22
# baseline (speedup 1.0000x reference)
"""DNN MVDR beamformer on 8 Trainium2 cores (one batch element per core).

V2 pipeline per core b (batch element b):
  A) x staged (t, th, c, f) bf16; masks (t, th, c, f) fp8 (cast-DMA'd to
     bf16 on load); channel-sum tree -> mb; sqm = sqrt(mb) (scalar);
     v = sqm * x (vector, 2x mode via c-outer f-inner layout)
  B) PSD: 3 matmuls per (fblock, chunk, mask) [Hermitian: ir = ri^T
     reconstructed later]; psum evacuated (scalar) -> sc -> DRAM scr
  C) P gather (affine diag-gather); imag = ri - ri^T (strided vector);
     attention -> u
  D) Gauss-Jordan solve (P_n + eps I) X = P_s, frequencies on partitions
  E) w = X@u / tr(X); bf16 block-diag weights via DRAM roundtrip
  F) beamform from fully-resident bf16 xfc tiles (prefetched on the
     gpsimd queue during A/B)
DMA queues: sync = x/scr/gathers; scalar = outputs/misc; gpsimd = masks+xfc.
"""
import sys, os
sys.path.insert(0, "/opt/trn_rl_repo")
from contextlib import ExitStack

import numpy as np
import ml_dtypes

import concourse.bass as bass
import concourse.mybir as mybir
import concourse.tile as tile
from concourse import bacc
from concourse.bass import _add_dep_helper
from concourse.bass_utils import run_bass_kernel_spmd

F32 = mybir.dt.float32
BF16 = mybir.dt.bfloat16
F8 = mybir.dt.float8e4
AX = mybir.AxisListType
OP = mybir.AluOpType
ACT = mybir.ActivationFunctionType

B, T, C, F, A = 8, 800, 8, 513, 320
FP = 528            # padded F (33 blocks of 16)
NFB = FP // 16      # 33
NST = 5             # f-partition stacks: f = st*128 + p (st4 holds 16 f's)
TCH = [(i * 128, min(128, T - i * 128)) for i in range(7)]
NTH = 3             # f thirds for phase A/B SBUF residency
FTH = FP // NTH     # 176
FBTH = NFB // NTH   # 11
EPS_MVDR = 1e-15
PAD_EPS = 1e-30
SCALING = 2.0
FBS = 128 * 512 + 128    # scratch per-fblock stride (affine-gather pad)
STS = 8 * FBS
RES_FB = 13              # xfc fblocks resident through A/B; rest stream in F


def build_nc():
    nc = bacc.Bacc("TRN2", target_bir_lowering=False, debug=False)

    xtc_r = nc.dram_tensor("xtc_r", [T, C * FP], BF16, kind="ExternalInput").ap()
    xtc_i = nc.dram_tensor("xtc_i", [T, C * FP], BF16, kind="ExternalInput").ap()
    xfc_r = nc.dram_tensor("xfc_r", [NFB * 128, T], BF16, kind="ExternalInput").ap()
    xfc_i = nc.dram_tensor("xfc_i", [NFB * 128, T], BF16, kind="ExternalInput").ap()
    mask_s = nc.dram_tensor("mask_s", [T, C * FP], BF16, kind="ExternalInput").ap()
    mask_n = nc.dram_tensor("mask_n", [T, C * FP], BF16, kind="ExternalInput").ap()
    mlp_w = nc.dram_tensor("mlp_w", [NST * 128, A], BF16, kind="ExternalInput").ap()
    mlp_b = nc.dram_tensor("mlp_b", [1, A], F32, kind="ExternalInput").ap()
    gvec_w = nc.dram_tensor("gvec_w", [1, A], F32, kind="ExternalInput").ap()
    gvec_b = nc.dram_tensor("gvec_b", [1, 1], F32, kind="ExternalInput").ap()
    ones_bf = nc.dram_tensor("ones_bf", [128, 1], BF16, kind="ExternalInput").ap()
    wzero = nc.dram_tensor("wzero", [1, 64], BF16, kind="ExternalInput").ap()

    out_dev = nc.dram_tensor("out_dev", [9 * 128, T], F32, kind="ExternalOutput").ap()

    scr = nc.dram_tensor("scr", [NFB * FBS], F32).ap()
    d_dram = nc.dram_tensor("d_dram", [NST * 128], F32).ap()
    e_dram = nc.dram_tensor("e_dram", [C], F32).ap()
    u_dram = nc.dram_tensor("u_dram", [C], F32).ap()
    w12_dram = nc.dram_tensor("w12_dram", [NST * 128 * 32], BF16).ap()

    with tile.TileContext(nc) as tc, ExitStack() as octx:
        spool = octx.enter_context(tc.tile_pool(name="sp", bufs=1))
        xfcp = octx.enter_context(tc.tile_pool(name="xfcp", bufs=1))

        P_all = spool.tile([128, NST * 4 * 64], F32, tag="P_all")
        nc.scalar.memzero(P_all[:])
        Pri = spool.tile([128, NST * 2 * 64], F32, tag="Pri")
        nc.scalar.memzero(Pri[:])
        # zeroed pad rows are safe: mag=0 there, so feat = 0 regardless of dk
        d_sb = spool.tile([1, NST * 128], F32, tag="dsb")
        nc.scalar.memzero(d_sb[:])

        xfr = {fb: xfcp.tile([128, T], BF16, tag=f"xfr{fb}", name=f"xfr{fb}")
               for fb in range(RES_FB)}
        xfi = {fb: xfcp.tile([128, T], BF16, tag=f"xfi{fb}", name=f"xfi{fb}")
               for fb in range(RES_FB)}

        scr_dumps = []
        ddump = None

        def gather_stack(st):
            # combs 0,2 (rr) -> P_all quarters 0,2; combs 1,3 (ri raw) -> Pri
            pcnt = 128 if st < 4 else 16
            for comb in range(4):
                gsrc = bass.AP(scr.tensor, st * STS + comb * 128,
                               [[4104, pcnt], [512, 8], [1, 8]])
                if comb % 2 == 0:
                    gdst = P_all[0:pcnt,
                                 (st * 4 + comb) * 64:(st * 4 + comb) * 64 + 64]
                else:
                    mi = comb // 2
                    gdst = Pri[0:pcnt,
                               (st * 2 + mi) * 64:(st * 2 + mi) * 64 + 64]
                geng = nc.sync if comb % 2 == 0 else nc.scalar
                g = geng.dma_start(gdst.rearrange("p (c e) -> p c e", c=8), gsrc)
                for dmp in scr_dumps[st * 8:st * 8 + 8]:
                    _add_dep_helper(g.ins, dmp.ins, reason="scr RAW")

        # ================= phases A+B =================
        with ExitStack() as ctx:
            vp = ctx.enter_context(tc.tile_pool(name="vp", bufs=1))
            xp = ctx.enter_context(tc.tile_pool(name="xp", bufs=2))
            mp = ctx.enter_context(tc.tile_pool(name="mp", bufs=2))
            scat = ctx.enter_context(tc.tile_pool(name="scat", bufs=2))
            pps = ctx.enter_context(tc.tile_pool(name="pps", bufs=2, space="PSUM"))
            ppd = ctx.enter_context(tc.tile_pool(name="ppd", bufs=1, space="PSUM"))

            ones_t = spool.tile([128, 1], BF16, tag="ones")
            nc.sync.dma_start(ones_t[:], ones_bf)
            d_ps0 = ppd.tile([1, 512], F32, tag="dps0")
            d_ps1 = ppd.tile([1, 128], F32, tag="dps1")
            d_mms = []
            xfc_emit = 0

            for th in range(NTH):
                f0 = th * FTH
                v = {}
                for ci, (t0, tn) in enumerate(TCH):
                    xh = {}
                    for comp, xsrc in ((0, xtc_r), (1, xtc_i)):
                        xt = xp.tile([128, 8 * FTH], BF16, tag=f"xh{comp}",
                                     name=f"xh{comp}_{th}_{ci}")
                        eng = nc.sync if comp == 0 else nc.scalar
                        eng.dma_start(
                            xt[:tn], xsrc[t0:t0 + tn, f0 * 8:(f0 + FTH) * 8])
                        xh[comp] = xt
                    for mi, msrc in enumerate((mask_s, mask_n)):
                        mk = mp.tile([128, 8 * FTH], BF16, tag=f"mk{mi}",
                                     name=f"mk{mi}_{th}_{ci}")
                        if mi == 0:
                            meng = nc.gpsimd
                        else:
                            meng = nc.sync if ci % 2 == 0 else nc.scalar
                        meng.dma_start(
                            mk[:tn], msrc[t0:t0 + tn, f0 * 8:(f0 + FTH) * 8])
                        mkv = mk[:tn].rearrange("t (f c) -> t f c", c=8)
                        # channel-sum tree folded in place into mk's low c's
                        l1v = mkv[:, :, 0:4]
                        nc.vector.tensor_add(l1v, mkv[:, :, 0:4], mkv[:, :, 4:8])
                        l2v = mkv[:, :, 0:2]
                        nc.vector.tensor_add(l2v, l1v[:, :, 0:2], l1v[:, :, 2:4])
                        mb = mp.tile([128, FTH], BF16, tag=f"mb{mi}",
                                     name=f"mb{mi}_{th}_{ci}")
                        nc.vector.tensor_add(mb[:tn], l2v[:, :, 0], l2v[:, :, 1])
                        if mi == 0:
                            first = (th == 0 and ci == 0)
                            last = (th == NTH - 1 and ci == len(TCH) - 1)
                            lo, hi = f0, f0 + FTH
                            if hi <= 512:
                                d_mms.append(nc.tensor.matmul(
                                    d_ps0[:, lo:hi], ones_t[:tn], mb[:tn],
                                    start=first, stop=last))
                            else:
                                d_mms.append(nc.tensor.matmul(
                                    d_ps0[:, lo:512], ones_t[:tn],
                                    mb[:tn, 0:512 - lo], start=first, stop=last))
                                d_mms.append(nc.tensor.matmul(
                                    d_ps1[:, 0:hi - 512], ones_t[:tn],
                                    mb[:tn, 512 - lo:], start=(ci == 0),
                                    stop=last))
                        mexp = mp.tile([128, 8 * FTH], BF16, tag=f"sq{mi}",
                                       name=f"sq{mi}_{th}_{ci}")
                        mbb = mb[:tn].unsqueeze(2).broadcast_to([tn, FTH, 8])
                        nc.scalar.activation(
                            mexp[:tn].rearrange("t (f c) -> t f c", c=8),
                            mbb, ACT.Sqrt)
                        for comp in range(2):
                            vt = vp.tile([128, 8 * FTH], BF16,
                                         tag=f"v{mi}{comp}_{ci}",
                                         name=f"v{mi}{comp}_{th}_{ci}")
                            nc.vector.tensor_mul(vt[:tn], xh[comp][:tn],
                                                 mexp[:tn])
                            v[mi, comp, ci] = vt
                    # drip xfc prefetch on the gpsimd queue behind the masks
                    want = max(0, th * 7 + ci - 6) * (2 * RES_FB) // 14
                    while xfc_emit < want:
                        fb2, comp2 = divmod(xfc_emit, 2)
                        dstt = xfr[fb2] if comp2 == 0 else xfi[fb2]
                        srct = xfc_r if comp2 == 0 else xfc_i
                        nc.gpsimd.dma_start(
                            dstt[:], srct[fb2 * 128:(fb2 + 1) * 128, :])
                        xfc_emit += 1

                def msl(tile_, tn, fo):
                    return tile_[:tn, fo * 8:fo * 8 + 128]

                nch = len(TCH)
                for g0 in range(0, FBTH, 3):
                    grp = list(range(g0, min(g0 + 3, FBTH)))
                    # one PSUM bank per fblock: cols [s_r | s_ri | n_r | n_ri]
                    ps = {}
                    for sl, fbl in enumerate(grp):
                        ps[fbl] = pps.tile([128, 512], F32, tag=f"ps{sl}",
                                           name=f"ps{sl}")
                    for ci, (t0, tn) in enumerate(TCH):
                        en_ = (ci == nch - 1)
                        for fbl in grp:
                            fo = 16 * fbl
                            for mi in range(2):
                                # ONE start=True per psum tile per round: the
                                # pending-zero mark is bank-granular (2KB), so
                                # per-slice starts would wipe sibling slices'
                                # first-chunk accumulation.
                                st_ = (ci == 0 and mi == 0)
                                vr = msl(v[mi, 0, ci], tn, fo)
                                vi = msl(v[mi, 1, ci], tn, fo)
                                pr = ps[fbl][:, mi * 256:mi * 256 + 128]
                                pri_ = ps[fbl][:, mi * 256 + 128:mi * 256 + 256]
                                nc.tensor.matmul(pr, vr, vr,
                                                 start=st_, stop=False,
                                                 skip_group_check=True)
                                nc.tensor.matmul(pr, vi, vi,
                                                 start=False, stop=en_,
                                                 skip_group_check=True)
                                nc.tensor.matmul(pri_, vi, vr,
                                                 start=False, stop=en_,
                                                 skip_group_check=True)
                    for sl, fbl in enumerate(grp):
                        fb = th * FBTH + fbl
                        sc = scat.tile([128, 512], F32, tag=f"sc{sl}",
                                       name=f"sc{sl}")
                        nc.scalar.copy(sc[:], ps[fbl][:])
                        scr_dumps.append(nc.sync.dma_start(
                            bass.AP(scr.tensor, fb * FBS, [[512, 128], [1, 512]]),
                            sc[:]))
                        if fb in (7, 15, 23, 31, 32):
                            gather_stack(fb // 8 if fb != 32 else 4)

            cpd0 = nc.scalar.copy(d_sb[:, 0:512], d_ps0[:])
            cpd1 = nc.scalar.copy(d_sb[:, 512:528], d_ps1[:, 0:16])
            for mm in d_mms:
                _add_dep_helper(cpd0.ins, mm.ins, reason="dps")
                _add_dep_helper(cpd1.ins, mm.ins, reason="dps")
            ddump = nc.sync.dma_start(
                bass.AP(d_dram.tensor, 0, [[640, 1], [1, 640]]), d_sb[:])

        # imag quarters: P_i = ri - ri^T (c<->e swap is a free-dim stride swap)
        pav = P_all[:].rearrange("p (s k c e) -> p s k c e", s=NST, k=4, c=8)
        priv = Pri[:].rearrange("p (s m c e) -> p s m c e", s=NST, m=2, c=8)
        privT = Pri[:].rearrange("p (s m e c) -> p s m c e", s=NST, m=2, e=8)
        pri_subs = []
        for mi in range(2):
            pri_subs.append(nc.vector.tensor_sub(
                pav[:, :, 2 * mi + 1, :, :],
                priv[:, :, mi, :, :], privT[:, :, mi, :, :]))

        d_f = spool.tile([128, NST], F32, tag="dfp")
        g = nc.sync.dma_start(
            d_f[:], bass.AP(d_dram.tensor, 0, [[1, 128], [128, NST]]))
        _add_dep_helper(g.ins, ddump.ins, reason="d RAW")

        # ================= phase C: attention =================
        with ExitStack() as ctx:
            ppa = ctx.enter_context(tc.tile_pool(name="ppa", bufs=1, space="PSUM"))
            rs = spool.tile([128, NST * 2 * 8], F32, tag="rs")
            rsv = rs[:].rearrange("p (s k c) -> p s k c", s=NST, k=2)
            for kk in range(2):
                nc.vector.tensor_reduce(rsv[:, :, kk, :], pav[:, :, kk, :, :],
                                        axis=AX.X, op=OP.add)
            Pbase = P_all[:].ap[0][0]
            diag_r = bass.AP(P_all.tensor, P_all.offset,
                             [[Pbase, 128], [256, NST], [64, 2], [9, 8]])
            nc.vector.tensor_sub(rsv, rsv, diag_r)
            sq = spool.tile([128, NST * 8], F32, tag="sq")
            sqv = sq[:].rearrange("p (s c) -> p s c", s=NST)
            nc.vector.tensor_mul(sqv, rsv[:, :, 0, :], rsv[:, :, 0, :])
            t2 = spool.tile([128, NST * 8], F32, tag="t2")
            t2v = t2[:].rearrange("p (s c) -> p s c", s=NST)
            nc.vector.tensor_mul(t2v, rsv[:, :, 1, :], rsv[:, :, 1, :])
            nc.vector.tensor_add(sqv, sqv, t2v)
            mag = spool.tile([128, NST * 8], F32, tag="mag")
            nc.scalar.activation(mag[:], sq[:], ACT.Sqrt)
            nc.vector.tensor_scalar_add(mag[:], mag[:], 1e-30)
            yr = spool.tile([128, NST * 8], F32, tag="yr")
            nc.vector.reciprocal(yr[:], mag[:])
            nc.vector.tensor_mul(yr[:], yr[:], sq[:])
            nc.vector.tensor_add(mag[:], mag[:], yr[:])
            dk = spool.tile([128, NST], F32, tag="dk")
            nc.vector.tensor_scalar_add(dk[:], d_f[:], 8 * EPS_MVDR)
            nc.vector.reciprocal(dk[:], dk[:])
            nc.vector.tensor_scalar_mul(dk[:], dk[:], 0.5 / 7.0)
            feat = spool.tile([128, NST * 8], BF16, tag="feat")
            featv = feat[:].rearrange("p (s c) -> p s c", s=NST)
            dkb = dk[:].unsqueeze(2).broadcast_to([128, NST, 8])
            magv = mag[:].rearrange("p (s c) -> p s c", s=NST)
            nc.vector.tensor_mul(featv, magv, dkb)
            mwt = spool.tile([128, NST * A], BF16, tag="mwt")
            for st in range(NST):
                nc.scalar.dma_start(mwt[:, st * A:(st + 1) * A],
                                    mlp_w[st * 128:(st + 1) * 128, :])
            hp = ppa.tile([8, A], F32, tag="hp")
            for st in range(NST):
                nc.tensor.matmul(hp[:], featv[:, st, :],
                                 mwt[:, st * A:(st + 1) * A],
                                 start=(st == 0), stop=(st == NST - 1))
            bias_t = spool.tile([8, A], F32, tag="bias")
            nc.scalar.dma_start(bias_t[:], bass.AP(mlp_b.tensor, 0, [[0, 8], [1, A]]))
            h = spool.tile([8, A], F32, tag="h")
            nc.vector.tensor_add(h[:], hp[:], bias_t[:])
            nc.scalar.activation(h[:], h[:], ACT.Tanh)
            gv = spool.tile([8, A], F32, tag="gv")
            nc.scalar.dma_start(gv[:], bass.AP(gvec_w.tensor, 0, [[0, 8], [1, A]]))
            nc.vector.tensor_mul(h[:], h[:], gv[:])
            ei = spool.tile([8, 1], F32, tag="ei")
            nc.vector.tensor_reduce(ei[:], h[:], axis=AX.X, op=OP.add)
            gb = spool.tile([8, 1], F32, tag="gb")
            nc.scalar.dma_start(gb[:], bass.AP(gvec_b.tensor, 0, [[0, 8], [1, 1]]))
            nc.vector.tensor_add(ei[:], ei[:], gb[:])
            edump = nc.sync.dma_start(
                bass.AP(e_dram.tensor, 0, [[1, 8], [0, 1]]), ei[:])
            erow = spool.tile([1, 8], F32, tag="erow")
            g = nc.sync.dma_start(erow[:],
                                  bass.AP(e_dram.tensor, 0, [[0, 1], [1, 8]]))
            _add_dep_helper(g.ins, edump.ins, reason="e RAW")
            emax = spool.tile([1, 1], F32, tag="emax")
            nc.vector.tensor_reduce(emax[:], erow[:], axis=AX.X, op=OP.max)
            nc.vector.tensor_scalar_mul(emax[:], emax[:], -SCALING)
            ex = spool.tile([1, 8], F32, tag="ex")
            nc.scalar.activation(ex[:], erow[:], ACT.Exp, bias=emax[:, 0:1],
                                 scale=SCALING)
            esum = spool.tile([1, 1], F32, tag="esum")
            nc.vector.tensor_reduce(esum[:], ex[:], axis=AX.X, op=OP.add)
            nc.vector.reciprocal(esum[:], esum[:])
            nc.vector.tensor_scalar_mul(ex[:], ex[:], esum[:, 0:1])
            udump = nc.sync.dma_start(
                bass.AP(u_dram.tensor, 0, [[8, 1], [1, 8]]), ex[:])
            u_all = spool.tile([128, 8], F32, tag="uall")
            g = nc.sync.dma_start(u_all[:],
                                  bass.AP(u_dram.tensor, 0, [[0, 128], [1, 8]]))
            _add_dep_helper(g.ins, udump.ins, reason="u RAW")

        # ================= phase D: Gauss-Jordan =================
        G = spool.tile([128, NST * 8 * 2 * 16], F32, tag="G")
        Gv = G[:].rearrange("p (s r k c) -> p s r k c", s=NST, r=8, k=2)
        for k in range(2):
            nc.vector.tensor_copy(Gv[:, :, :, k, 0:8], pav[:, :, 2 + k, :, :])
            nc.vector.tensor_copy(Gv[:, :, :, k, 8:16], pav[:, :, k, :, :])
        Gbase = G[:].ap[0][0]
        diagA = bass.AP(G.tensor, G.offset, [[Gbase, 128], [256, NST], [33, 8]])
        nc.vector.tensor_scalar_add(diagA, diagA, PAD_EPS)
        recs = spool.tile([128, NST * 8], F32, tag="recs")
        recsv = recs[:].rearrange("p (s r) -> p s r", s=NST)
        fv = spool.tile([128, NST * 8 * 2], F32, tag="fv")
        fvv = fv[:].rearrange("p (s r k) -> p s r k", s=NST, r=8)
        tt = spool.tile([128, NST * 8 * 16], F32, tag="tt")
        ttv = tt[:].rearrange("p (s r c) -> p s r c", s=NST, r=8)
        for k in range(8):
            piv = bass.AP(G.tensor, G.offset + k * 32 + k,
                          [[Gbase, 128], [256, NST], [0, 1]])
            nc.vector.reciprocal(recsv[:, :, k:k + 1], piv)
            colk = Gv[:, :, :, :, k]
            rb = recsv[:, :, k:k + 1].unsqueeze(2).broadcast_to([128, NST, 8, 2])
            nc.vector.tensor_mul(fvv, colk, rb)
            nc.scalar.mul(fvv[:, :, k, :], fvv[:, :, k, :], 0.0)
            ncol = 16 - k
            sh = [128, NST, 8, ncol]
            grow_r = Gv[:, :, k, 0, k:16].unsqueeze(2).broadcast_to(sh)
            grow_i = Gv[:, :, k, 1, k:16].unsqueeze(2).broadcast_to(sh)
            fr = fvv[:, :, :, 0].unsqueeze(3).broadcast_to(sh)
            fi = fvv[:, :, :, 1].unsqueeze(3).broadcast_to(sh)
            tv = ttv[:, :, :, 0:ncol]
            Gr = Gv[:, :, :, 0, k:16]
            Gi = Gv[:, :, :, 1, k:16]
            nc.vector.tensor_mul(tv, fr, grow_r)
            nc.vector.tensor_sub(Gr, Gr, tv)
            nc.vector.tensor_mul(tv, fi, grow_i)
            nc.vector.tensor_add(Gr, Gr, tv)
            nc.vector.tensor_mul(tv, fr, grow_i)
            nc.vector.tensor_sub(Gi, Gi, tv)
            nc.vector.tensor_mul(tv, fi, grow_r)
            nc.vector.tensor_sub(Gi, Gi, tv)
        Xs = spool.tile([128, NST * 8 * 2 * 8], F32, tag="Xs")
        Xm = Xs[:].rearrange("p (sr k c) -> p sr k c", k=2, c=8)
        Gm = G[:].rearrange("p (sr k c) -> p sr k c", k=2, c=16)
        rb2 = recs[:].unsqueeze(2).unsqueeze(3).broadcast_to([128, NST * 8, 2, 8])
        nc.vector.tensor_mul(Xm, Gm[:, :, :, 8:16], rb2)

        # ================= phase E: trace, w, wbd =================
        Xbase = Xs[:].ap[0][0]
        trv = spool.tile([128, NST * 2], F32, tag="trv")
        trvv = trv[:].rearrange("p (s k) -> p s k", s=NST)
        diagX = bass.AP(Xs.tensor, Xs.offset,
                        [[Xbase, 128], [128, NST], [8, 2], [17, 8]])
        nc.vector.tensor_reduce(trvv, diagX, axis=AX.X, op=OP.add)
        nc.vector.tensor_scalar_add(trvv[:, :, 0:1], trvv[:, :, 0:1], EPS_MVDR)
        ub = u_all[:].unsqueeze(1).broadcast_to([128, NST * 8 * 2, 8])
        wtmp = spool.tile([128, NST * 8 * 2 * 8], F32, tag="wtmp")
        wtm = wtmp[:].rearrange("p (srk c) -> p srk c", c=8)
        Xm2 = Xs[:].rearrange("p (srk c) -> p srk c", c=8)
        nc.vector.tensor_mul(wtm, Xm2, ub)
        wraw = spool.tile([128, NST * 8 * 2], F32, tag="wraw")
        wrv = wraw[:].rearrange("p (s r k) -> p s r k", s=NST, r=8)
        nc.vector.tensor_reduce(wraw[:].unsqueeze(2), wtm, axis=AX.X, op=OP.add)
        t2m = spool.tile([128, NST], F32, tag="t2m")
        t2mv = t2m[:].unsqueeze(2)
        nc.vector.tensor_mul(t2mv, trvv[:, :, 0:1], trvv[:, :, 0:1])
        tmi = spool.tile([128, NST], F32, tag="tmi")
        tmiv = tmi[:].unsqueeze(2)
        nc.vector.tensor_mul(tmiv, trvv[:, :, 1:2], trvv[:, :, 1:2])
        nc.vector.tensor_add(t2m[:], t2m[:], tmi[:])
        nc.vector.reciprocal(t2m[:], t2m[:])
        ctr = spool.tile([128, NST * 2], F32, tag="ctr")
        ctrv = ctr[:].rearrange("p (s k) -> p s k", s=NST)
        t2b = t2m[:].unsqueeze(2).broadcast_to([128, NST, 2])
        nc.vector.tensor_mul(ctrv, trvv, t2b)
        nc.scalar.mul(ctrv[:, :, 1:2], ctrv[:, :, 1:2], -1.0)
        wf = spool.tile([128, NST * 8 * 2], F32, tag="wf")
        wfv = wf[:].rearrange("p (s r k) -> p s r k", s=NST, r=8)
        cr = ctrv[:, :, 0:1].unsqueeze(2).broadcast_to([128, NST, 8, 1])
        ci_ = ctrv[:, :, 1:2].unsqueeze(2).broadcast_to([128, NST, 8, 1])
        wr_tmp = spool.tile([128, NST * 8], F32, tag="wrt")
        wrtv = wr_tmp[:].rearrange("p (s r) -> p s r", s=NST).unsqueeze(3)
        nc.vector.tensor_mul(wfv[:, :, :, 0:1], wrv[:, :, :, 0:1], cr)
        nc.vector.tensor_mul(wrtv, wrv[:, :, :, 1:2], ci_)
        nc.vector.tensor_sub(wfv[:, :, :, 0:1], wfv[:, :, :, 0:1], wrtv)
        nc.vector.tensor_mul(wfv[:, :, :, 1:2], wrv[:, :, :, 1:2], cr)
        nc.vector.tensor_mul(wrtv, wrv[:, :, :, 0:1], ci_)
        nc.vector.tensor_add(wfv[:, :, :, 1:2], wfv[:, :, :, 1:2], wrtv)
        # w12: v0 = (wr, -wi) pairs x_r; v1 = (wi, wr) pairs x_i  (conj(w))
        w12 = spool.tile([128, NST * 32], BF16, tag="w12")
        w12v = w12[:].rearrange("p (s v c o) -> p s v c o", s=NST, v=2, c=8)
        nc.vector.tensor_copy(w12v[:, :, 0, :, 0:1], wfv[:, :, :, 0:1])
        nc.scalar.mul(w12v[:, :, 0, :, 1:2], wfv[:, :, :, 1:2], -1.0)
        nc.vector.tensor_copy(w12v[:, :, 1, :, 0:1], wfv[:, :, :, 1:2])
        nc.vector.tensor_copy(w12v[:, :, 1, :, 1:2], wfv[:, :, :, 0:1])
        wdumps = []
        for st in range(NST):
            d = nc.sync.dma_start(
                bass.AP(w12_dram.tensor, st * 128 * 32, [[32, 128], [1, 32]]),
                w12[:, st * 32:(st + 1) * 32])
            wdumps.append(d)
        wbd = spool.tile([128, NFB * 64], BF16, tag="wbd")
        nc.scalar.dma_start(wbd[:].rearrange("p (b e) -> p b e", b=NFB),
                            bass.AP(wzero.tensor, 0, [[0, 128], [0, NFB], [1, 64]]))
        wpitch = wbd[:].ap[0][0]
        for ft in range(16):
            for vv_ in range(2):
                src = bass.AP(w12_dram.tensor, ft * 32 + vv_ * 16,
                              [[2, 8], [512, NFB], [1, 2]])
                dst = bass.AP(wbd.tensor,
                              wbd.offset + (8 * ft) * wpitch + vv_ * 32 + ft * 2,
                              [[wpitch, 8], [64, NFB], [1, 2]])
                geng = nc.scalar if (ft % 2 == 0) else nc.sync
                g = geng.dma_start(dst, src)
                for d in wdumps:
                    _add_dep_helper(g.ins, d.ins, reason="w12 RAW")

        # ================= phase F: beamform =================
        with ExitStack() as ctx:
            outp = ctx.enter_context(tc.tile_pool(name="outp", bufs=2))
            xfs = ctx.enter_context(tc.tile_pool(name="xfs", bufs=2))
            ppb = ctx.enter_context(tc.tile_pool(name="ppb", bufs=2, space="PSUM"))
            next_load = RES_FB

            def stream_to(limit):
                nonlocal next_load
                while next_load < min(NFB, limit):
                    fb2 = next_load
                    xr_t = xfs.tile([128, T], BF16, tag=f"xsr{fb2 % 4}",
                                    name=f"xsr{fb2}")
                    xi_t = xfs.tile([128, T], BF16, tag=f"xsi{fb2 % 4}",
                                    name=f"xsi{fb2}")
                    nc.gpsimd.dma_start(xr_t[:],
                                        xfc_r[fb2 * 128:(fb2 + 1) * 128, :])
                    nc.sync.dma_start(xi_t[:],
                                      xfc_i[fb2 * 128:(fb2 + 1) * 128, :])
                    xfr[fb2], xfi[fb2] = xr_t, xi_t
                    next_load += 1

            stream_to(RES_FB + 4)
            for gi in range(9):
                nfb_g = 4 if gi < 8 else 1
                stream_to((gi + 2) * 4)
                for (t0, tn) in [(0, 512), (512, 288)]:
                    ob = outp.tile([128, 512], F32, tag="ob", name=f"ob{gi}")
                    for q in range(nfb_g):
                        fb = gi * 4 + q
                        bp = ppb.tile([32, 512], F32, tag=f"bps{q % 2}",
                                      name=f"bps{q}")
                        nc.tensor.matmul(bp[0:32, 0:tn],
                                         wbd[:, fb * 64:fb * 64 + 32],
                                         xfr[fb][:, t0:t0 + tn],
                                         start=True, stop=False)
                        nc.tensor.matmul(bp[0:32, 0:tn],
                                         wbd[:, fb * 64 + 32:fb * 64 + 64],
                                         xfi[fb][:, t0:t0 + tn],
                                         start=False, stop=True)
                        eng = nc.vector.tensor_copy if q % 2 == 0 else nc.scalar.copy
                        eng(ob[32 * q:32 * q + 32, 0:tn], bp[0:32, 0:tn])
                    nc.scalar.dma_start(
                        out_dev[gi * 128:gi * 128 + 32 * nfb_g, t0:t0 + tn],
                        ob[0:32 * nfb_g, 0:tn])

    nc.compile()
    return nc


_NC_CACHE = None


def _get_nc():
    global _NC_CACHE
    if _NC_CACHE is None:
        _NC_CACHE = build_nc()
    return _NC_CACHE


def make_core_inputs(data_real, data_imag, mask_speech, mask_noise,
                     mlp_w, mlp_b, gvec_w, gvec_b, b):
    bf16 = ml_dtypes.bfloat16
    f8 = ml_dtypes.float8_e4m3
    xr = np.zeros((T, C, FP), np.float32)
    xi = np.zeros((T, C, FP), np.float32)
    xr[:, :, :F] = data_real[b]
    xi[:, :, :F] = data_imag[b]
    ms = np.zeros((T, C, FP), np.float32)
    mn = np.zeros((T, C, FP), np.float32)
    ms[:, :, :F] = np.transpose(np.asarray(mask_speech[b]), (2, 1, 0))
    mn[:, :, :F] = np.transpose(np.asarray(mask_noise[b]), (2, 1, 0))
    mw = np.zeros((NST * 128, A), np.float32)
    mw[:F] = mlp_w

    def thc(a):  # (T, C, FP) -> (T, NTH*FTH*C) with th-outer, f, c-inner
        return np.ascontiguousarray(
            a.reshape(T, C, NTH, FTH).transpose(0, 2, 3, 1).reshape(T, C * FP))

    return dict(
        xtc_r=thc(xr).astype(bf16),
        xtc_i=thc(xi).astype(bf16),
        xfc_r=np.ascontiguousarray(
            xr.transpose(2, 1, 0).reshape(NFB * 128, T)).astype(bf16),
        xfc_i=np.ascontiguousarray(
            xi.transpose(2, 1, 0).reshape(NFB * 128, T)).astype(bf16),
        mask_s=thc(ms).astype(bf16),
        mask_n=thc(mn).astype(bf16),
        mlp_w=mw.astype(bf16),
        mlp_b=np.asarray(mlp_b).reshape(1, A).astype(np.float32),
        gvec_w=np.asarray(gvec_w).reshape(1, A).astype(np.float32),
        gvec_b=np.asarray(gvec_b).reshape(1, 1).astype(np.float32),
        ones_bf=np.ones((128, 1), bf16),
        wzero=np.zeros((1, 64), bf16),
    )


def assemble_output(core_outs):
    out = np.zeros((B, T, F, 2), np.float32)
    for b in range(B):
        od = np.asarray(core_outs[b]["out_dev"]).reshape(9 * 128, T)
        # row = gi*128 + q*32 + fprime*2 + oc ; f = 16*(4*gi+q) + fprime
        rows = od.reshape(9, 4, 16, 2, T)
        full = rows.reshape(36, 16, 2, T).reshape(576, 2, T)
        out[b, :, :, 0] = full[:F, 0, :].T
        out[b, :, :, 1] = full[:F, 1, :].T
    return out


def kernel(**inputs):
    nc = _get_nc()
    in_maps = [
        make_core_inputs(inputs["data_real"], inputs["data_imag"],
                         inputs["mask_speech"], inputs["mask_noise"],
                         inputs["mlp_w"], inputs["mlp_b"],
                         inputs["gvec_w"], inputs["gvec_b"], b)
        for b in range(B)
    ]
    res = run_bass_kernel_spmd(nc, in_maps, list(range(B))).results
    return assemble_output(res)


if __name__ == "__main__":
    build_nc()
    print("built ok")


# revision 23
# speedup vs baseline: 1.0642x; 1.0642x over previous
"""DNN MVDR beamformer on 8 Trainium2 cores (one batch element per core).

V2 pipeline per core b (batch element b):
  A) x staged (t, th, c, f) bf16; masks (t, th, c, f) fp8 (cast-DMA'd to
     bf16 on load); channel-sum tree -> mb; sqm = sqrt(mb) (scalar);
     v = sqm * x (vector, 2x mode via c-outer f-inner layout)
  B) PSD: 3 matmuls per (fblock, chunk, mask) [Hermitian: ir = ri^T
     reconstructed later]; psum evacuated (scalar) -> sc -> DRAM scr
  C) P gather (affine diag-gather); imag = ri - ri^T (strided vector);
     attention -> u
  D) Gauss-Jordan solve (P_n + eps I) X = P_s, frequencies on partitions
  E) w = X@u / tr(X); bf16 block-diag weights via DRAM roundtrip
  F) beamform from fully-resident bf16 xfc tiles (prefetched on the
     gpsimd queue during A/B)
DMA queues: sync = x/scr/gathers; scalar = outputs/misc; gpsimd = masks+xfc.
"""
import sys, os
sys.path.insert(0, "/opt/trn_rl_repo")
from contextlib import ExitStack

import numpy as np
import ml_dtypes

import concourse.bass as bass
import concourse.mybir as mybir
import concourse.tile as tile
from concourse import bacc
from concourse.bass import _add_dep_helper
from concourse.bass_utils import run_bass_kernel_spmd

F32 = mybir.dt.float32
BF16 = mybir.dt.bfloat16
F8 = mybir.dt.float8e4
AX = mybir.AxisListType
OP = mybir.AluOpType
ACT = mybir.ActivationFunctionType

B, T, C, F, A = 8, 800, 8, 513, 320
FP = 528            # padded F (33 blocks of 16)
NFB = FP // 16      # 33
NST = 5             # f-partition stacks: f = st*128 + p (st4 holds 16 f's)
TCH = [(i * 128, min(128, T - i * 128)) for i in range(7)]
NTH = 3             # f thirds for phase A/B SBUF residency
FTH = FP // NTH     # 176
FBTH = NFB // NTH   # 11
EPS_MVDR = 1e-15
PAD_EPS = 1e-30
SCALING = 2.0
FBS = 128 * 512 + 128    # scratch per-fblock stride (affine-gather pad)
STS = 8 * FBS
RES_FB = 13              # xfc fblocks resident through A/B; rest stream in F


def build_nc():
    nc = bacc.Bacc("TRN2", target_bir_lowering=False, debug=False)

    xtc_r = nc.dram_tensor("xtc_r", [T, C * FP], BF16, kind="ExternalInput").ap()
    xtc_i = nc.dram_tensor("xtc_i", [T, C * FP], BF16, kind="ExternalInput").ap()
    xfc_r = nc.dram_tensor("xfc_r", [NFB * 128, T], BF16, kind="ExternalInput").ap()
    xfc_i = nc.dram_tensor("xfc_i", [NFB * 128, T], BF16, kind="ExternalInput").ap()
    mask_s = nc.dram_tensor("mask_s", [T, C * FP], BF16, kind="ExternalInput").ap()
    mask_n = nc.dram_tensor("mask_n", [T, C * FP], BF16, kind="ExternalInput").ap()
    mlp_w = nc.dram_tensor("mlp_w", [NST * 128, A], BF16, kind="ExternalInput").ap()
    mlp_b = nc.dram_tensor("mlp_b", [1, A], F32, kind="ExternalInput").ap()
    gvec_w = nc.dram_tensor("gvec_w", [1, A], F32, kind="ExternalInput").ap()
    gvec_b = nc.dram_tensor("gvec_b", [1, 1], F32, kind="ExternalInput").ap()
    ones_bf = nc.dram_tensor("ones_bf", [128, 1], BF16, kind="ExternalInput").ap()
    wzero = nc.dram_tensor("wzero", [1, 64], BF16, kind="ExternalInput").ap()

    out_dev = nc.dram_tensor("out_dev", [9 * 128, T], F32, kind="ExternalOutput").ap()

    scr = nc.dram_tensor("scr", [NFB * FBS], F32).ap()
    d_dram = nc.dram_tensor("d_dram", [NST * 128], F32).ap()
    e_dram = nc.dram_tensor("e_dram", [C], F32).ap()
    u_dram = nc.dram_tensor("u_dram", [C], F32).ap()
    w12_dram = nc.dram_tensor("w12_dram", [NST * 128 * 32], BF16).ap()

    with tile.TileContext(nc) as tc, ExitStack() as octx:
        spool = octx.enter_context(tc.tile_pool(name="sp", bufs=1))
        xfcp = octx.enter_context(tc.tile_pool(name="xfcp", bufs=1))

        P_all = spool.tile([128, NST * 4 * 64], F32, tag="P_all")
        nc.gpsimd.memset(P_all[:], 0.0)
        Pri = spool.tile([128, NST * 2 * 64], F32, tag="Pri")
        nc.gpsimd.memset(Pri[:], 0.0)
        d_sb = spool.tile([1, NST * 128], F32, tag="dsb")
        nc.gpsimd.memset(d_sb[:], 1.0)

        xfr = {fb: xfcp.tile([128, T], BF16, tag=f"xfr{fb}", name=f"xfr{fb}")
               for fb in range(RES_FB)}
        xfi = {fb: xfcp.tile([128, T], BF16, tag=f"xfi{fb}", name=f"xfi{fb}")
               for fb in range(RES_FB)}

        scr_dumps = []
        ddump = None

        def gather_stack(st):
            # combs 0,2 (rr) -> P_all quarters 0,2; combs 1,3 (ri raw) -> Pri
            pcnt = 128 if st < 4 else 16
            for comb in range(4):
                gsrc = bass.AP(scr.tensor, st * STS + comb * 128,
                               [[4104, pcnt], [512, 8], [1, 8]])
                if comb % 2 == 0:
                    gdst = P_all[0:pcnt,
                                 (st * 4 + comb) * 64:(st * 4 + comb) * 64 + 64]
                else:
                    mi = comb // 2
                    gdst = Pri[0:pcnt,
                               (st * 2 + mi) * 64:(st * 2 + mi) * 64 + 64]
                geng = nc.sync if comb % 2 == 0 else nc.scalar
                g = geng.dma_start(gdst.rearrange("p (c e) -> p c e", c=8), gsrc)
                for dmp in scr_dumps[st * 8:st * 8 + 8]:
                    _add_dep_helper(g.ins, dmp.ins, reason="scr RAW")

        # ================= phases A+B =================
        with ExitStack() as ctx:
            vp = ctx.enter_context(tc.tile_pool(name="vp", bufs=1))
            xp = ctx.enter_context(tc.tile_pool(name="xp", bufs=2))
            mp = ctx.enter_context(tc.tile_pool(name="mp", bufs=2))
            scat = ctx.enter_context(tc.tile_pool(name="scat", bufs=2))
            pps = ctx.enter_context(tc.tile_pool(name="pps", bufs=2, space="PSUM"))
            ppd = ctx.enter_context(tc.tile_pool(name="ppd", bufs=1, space="PSUM"))

            ones_t = spool.tile([128, 1], BF16, tag="ones")
            nc.sync.dma_start(ones_t[:], ones_bf)
            d_ps0 = ppd.tile([1, 512], F32, tag="dps0")
            d_ps1 = ppd.tile([1, 128], F32, tag="dps1")
            d_mms = []
            xfc_emit = 0

            for th in range(NTH):
                f0 = th * FTH
                v = {}
                for ci, (t0, tn) in enumerate(TCH):
                    xh = {}
                    for comp, xsrc in ((0, xtc_r), (1, xtc_i)):
                        xt = xp.tile([128, 8 * FTH], BF16, tag=f"xh{comp}",
                                     name=f"xh{comp}_{th}_{ci}")
                        eng = nc.sync if comp == 0 else nc.scalar
                        eng.dma_start(
                            xt[:tn], xsrc[t0:t0 + tn, f0 * 8:(f0 + FTH) * 8])
                        xh[comp] = xt
                    for mi, msrc in enumerate((mask_s, mask_n)):
                        mk = mp.tile([128, 8 * FTH], BF16, tag=f"mk{mi}",
                                     name=f"mk{mi}_{th}_{ci}")
                        nc.gpsimd.dma_start(
                            mk[:tn], msrc[t0:t0 + tn, f0 * 8:(f0 + FTH) * 8])
                        mkv = mk[:tn].rearrange("t (f c) -> t f c", c=8)
                        # channel-sum tree folded in place into mk's low c's
                        l1v = mkv[:, :, 0:4]
                        nc.vector.tensor_add(l1v, mkv[:, :, 0:4], mkv[:, :, 4:8])
                        l2v = mkv[:, :, 0:2]
                        nc.vector.tensor_add(l2v, l1v[:, :, 0:2], l1v[:, :, 2:4])
                        mb = mp.tile([128, FTH], BF16, tag=f"mb{mi}",
                                     name=f"mb{mi}_{th}_{ci}")
                        nc.vector.tensor_add(mb[:tn], l2v[:, :, 0], l2v[:, :, 1])
                        if mi == 0:
                            first = (th == 0 and ci == 0)
                            last = (th == NTH - 1 and ci == len(TCH) - 1)
                            lo, hi = f0, f0 + FTH
                            if hi <= 512:
                                d_mms.append(nc.tensor.matmul(
                                    d_ps0[:, lo:hi], ones_t[:tn], mb[:tn],
                                    start=first, stop=last))
                            else:
                                d_mms.append(nc.tensor.matmul(
                                    d_ps0[:, lo:512], ones_t[:tn],
                                    mb[:tn, 0:512 - lo], start=first, stop=last))
                                d_mms.append(nc.tensor.matmul(
                                    d_ps1[:, 0:hi - 512], ones_t[:tn],
                                    mb[:tn, 512 - lo:], start=(ci == 0),
                                    stop=last))
                        mexp = mp.tile([128, 8 * FTH], BF16, tag=f"sq{mi}",
                                       name=f"sq{mi}_{th}_{ci}")
                        mbb = mb[:tn].unsqueeze(2).broadcast_to([tn, FTH, 8])
                        nc.scalar.activation(
                            mexp[:tn].rearrange("t (f c) -> t f c", c=8),
                            mbb, ACT.Sqrt)
                        for comp in range(2):
                            vt = vp.tile([128, 8 * FTH], BF16,
                                         tag=f"v{mi}{comp}_{ci}",
                                         name=f"v{mi}{comp}_{th}_{ci}")
                            nc.vector.tensor_mul(vt[:tn], xh[comp][:tn],
                                                 mexp[:tn])
                            v[mi, comp, ci] = vt
                    # drip xfc prefetch on the gpsimd queue behind the masks
                    want = max(0, th * 7 + ci - 6) * (2 * RES_FB) // 14
                    while xfc_emit < want:
                        fb2, comp2 = divmod(xfc_emit, 2)
                        dstt = xfr[fb2] if comp2 == 0 else xfi[fb2]
                        srct = xfc_r if comp2 == 0 else xfc_i
                        nc.gpsimd.dma_start(
                            dstt[:], srct[fb2 * 128:(fb2 + 1) * 128, :])
                        xfc_emit += 1

                def msl(tile_, tn, fo):
                    return tile_[:tn, fo * 8:fo * 8 + 128]

                nch = len(TCH)
                for g0 in range(0, FBTH, 3):
                    grp = list(range(g0, min(g0 + 3, FBTH)))
                    # one PSUM bank per fblock: cols [s_r | s_ri | n_r | n_ri]
                    ps = {}
                    for sl, fbl in enumerate(grp):
                        ps[fbl] = pps.tile([128, 512], F32, tag=f"ps{sl}",
                                           name=f"ps{sl}")
                    for ci, (t0, tn) in enumerate(TCH):
                        en_ = (ci == nch - 1)
                        for fbl in grp:
                            fo = 16 * fbl
                            for mi in range(2):
                                # ONE start=True per psum tile per round: the
                                # pending-zero mark is bank-granular (2KB), so
                                # per-slice starts would wipe sibling slices'
                                # first-chunk accumulation.
                                st_ = (ci == 0 and mi == 0)
                                vr = msl(v[mi, 0, ci], tn, fo)
                                vi = msl(v[mi, 1, ci], tn, fo)
                                pr = ps[fbl][:, mi * 256:mi * 256 + 128]
                                pri_ = ps[fbl][:, mi * 256 + 128:mi * 256 + 256]
                                nc.tensor.matmul(pr, vr, vr,
                                                 start=st_, stop=False,
                                                 skip_group_check=True)
                                nc.tensor.matmul(pr, vi, vi,
                                                 start=False, stop=en_,
                                                 skip_group_check=True)
                                nc.tensor.matmul(pri_, vi, vr,
                                                 start=False, stop=en_,
                                                 skip_group_check=True)
                    for sl, fbl in enumerate(grp):
                        fb = th * FBTH + fbl
                        sc = scat.tile([128, 512], F32, tag=f"sc{sl}",
                                       name=f"sc{sl}")
                        nc.scalar.copy(sc[:], ps[fbl][:])
                        scr_dumps.append(nc.sync.dma_start(
                            bass.AP(scr.tensor, fb * FBS, [[512, 128], [1, 512]]),
                            sc[:]))
                        if fb in (7, 15, 23, 31, 32):
                            gather_stack(fb // 8 if fb != 32 else 4)

            cpd0 = nc.scalar.copy(d_sb[:, 0:512], d_ps0[:])
            cpd1 = nc.scalar.copy(d_sb[:, 512:528], d_ps1[:, 0:16])
            for mm in d_mms:
                _add_dep_helper(cpd0.ins, mm.ins, reason="dps")
                _add_dep_helper(cpd1.ins, mm.ins, reason="dps")
            ddump = nc.sync.dma_start(
                bass.AP(d_dram.tensor, 0, [[640, 1], [1, 640]]), d_sb[:])

        # imag quarters: P_i = ri - ri^T (c<->e swap is a free-dim stride swap)
        pav = P_all[:].rearrange("p (s k c e) -> p s k c e", s=NST, k=4, c=8)
        priv = Pri[:].rearrange("p (s m c e) -> p s m c e", s=NST, m=2, c=8)
        privT = Pri[:].rearrange("p (s m e c) -> p s m c e", s=NST, m=2, e=8)
        pri_subs = []
        for mi in range(2):
            pri_subs.append(nc.vector.tensor_sub(
                pav[:, :, 2 * mi + 1, :, :],
                priv[:, :, mi, :, :], privT[:, :, mi, :, :]))

        d_f = spool.tile([128, NST], F32, tag="dfp")
        g = nc.sync.dma_start(
            d_f[:], bass.AP(d_dram.tensor, 0, [[1, 128], [128, NST]]))
        _add_dep_helper(g.ins, ddump.ins, reason="d RAW")

        # ================= phase C: attention =================
        with ExitStack() as ctx:
            ppa = ctx.enter_context(tc.tile_pool(name="ppa", bufs=1, space="PSUM"))
            rs = spool.tile([128, NST * 2 * 8], F32, tag="rs")
            rsv = rs[:].rearrange("p (s k c) -> p s k c", s=NST, k=2)
            for kk in range(2):
                nc.vector.tensor_reduce(rsv[:, :, kk, :], pav[:, :, kk, :, :],
                                        axis=AX.X, op=OP.add)
            Pbase = P_all[:].ap[0][0]
            diag_r = bass.AP(P_all.tensor, P_all.offset,
                             [[Pbase, 128], [256, NST], [64, 2], [9, 8]])
            nc.vector.tensor_sub(rsv, rsv, diag_r)
            sq = spool.tile([128, NST * 8], F32, tag="sq")
            sqv = sq[:].rearrange("p (s c) -> p s c", s=NST)
            nc.vector.tensor_mul(sqv, rsv[:, :, 0, :], rsv[:, :, 0, :])
            t2 = spool.tile([128, NST * 8], F32, tag="t2")
            t2v = t2[:].rearrange("p (s c) -> p s c", s=NST)
            nc.vector.tensor_mul(t2v, rsv[:, :, 1, :], rsv[:, :, 1, :])
            nc.vector.tensor_add(sqv, sqv, t2v)
            mag = spool.tile([128, NST * 8], F32, tag="mag")
            nc.scalar.activation(mag[:], sq[:], ACT.Sqrt)
            nc.vector.tensor_scalar_add(mag[:], mag[:], 1e-30)
            yr = spool.tile([128, NST * 8], F32, tag="yr")
            nc.vector.reciprocal(yr[:], mag[:])
            nc.vector.tensor_mul(yr[:], yr[:], sq[:])
            nc.vector.tensor_add(mag[:], mag[:], yr[:])
            dk = spool.tile([128, NST], F32, tag="dk")
            nc.vector.tensor_scalar_add(dk[:], d_f[:], 8 * EPS_MVDR)
            nc.vector.reciprocal(dk[:], dk[:])
            nc.vector.tensor_scalar_mul(dk[:], dk[:], 0.5 / 7.0)
            feat = spool.tile([128, NST * 8], BF16, tag="feat")
            featv = feat[:].rearrange("p (s c) -> p s c", s=NST)
            dkb = dk[:].unsqueeze(2).broadcast_to([128, NST, 8])
            magv = mag[:].rearrange("p (s c) -> p s c", s=NST)
            nc.vector.tensor_mul(featv, magv, dkb)
            mwt = spool.tile([128, NST * A], BF16, tag="mwt")
            for st in range(NST):
                nc.scalar.dma_start(mwt[:, st * A:(st + 1) * A],
                                    mlp_w[st * 128:(st + 1) * 128, :])
            hp = ppa.tile([8, A], F32, tag="hp")
            for st in range(NST):
                nc.tensor.matmul(hp[:], featv[:, st, :],
                                 mwt[:, st * A:(st + 1) * A],
                                 start=(st == 0), stop=(st == NST - 1))
            bias_t = spool.tile([8, A], F32, tag="bias")
            nc.scalar.dma_start(bias_t[:], bass.AP(mlp_b.tensor, 0, [[0, 8], [1, A]]))
            h = spool.tile([8, A], F32, tag="h")
            nc.vector.tensor_add(h[:], hp[:], bias_t[:])
            nc.scalar.activation(h[:], h[:], ACT.Tanh)
            gv = spool.tile([8, A], F32, tag="gv")
            nc.scalar.dma_start(gv[:], bass.AP(gvec_w.tensor, 0, [[0, 8], [1, A]]))
            nc.vector.tensor_mul(h[:], h[:], gv[:])
            ei = spool.tile([8, 1], F32, tag="ei")
            nc.vector.tensor_reduce(ei[:], h[:], axis=AX.X, op=OP.add)
            gb = spool.tile([8, 1], F32, tag="gb")
            nc.scalar.dma_start(gb[:], bass.AP(gvec_b.tensor, 0, [[0, 8], [1, 1]]))
            nc.vector.tensor_add(ei[:], ei[:], gb[:])
            edump = nc.sync.dma_start(
                bass.AP(e_dram.tensor, 0, [[1, 8], [0, 1]]), ei[:])
            erow = spool.tile([1, 8], F32, tag="erow")
            g = nc.sync.dma_start(erow[:],
                                  bass.AP(e_dram.tensor, 0, [[0, 1], [1, 8]]))
            _add_dep_helper(g.ins, edump.ins, reason="e RAW")
            emax = spool.tile([1, 1], F32, tag="emax")
            nc.vector.tensor_reduce(emax[:], erow[:], axis=AX.X, op=OP.max)
            nc.vector.tensor_scalar_mul(emax[:], emax[:], -SCALING)
            ex = spool.tile([1, 8], F32, tag="ex")
            nc.scalar.activation(ex[:], erow[:], ACT.Exp, bias=emax[:, 0:1],
                                 scale=SCALING)
            esum = spool.tile([1, 1], F32, tag="esum")
            nc.vector.tensor_reduce(esum[:], ex[:], axis=AX.X, op=OP.add)
            nc.vector.reciprocal(esum[:], esum[:])
            nc.vector.tensor_scalar_mul(ex[:], ex[:], esum[:, 0:1])
            udump = nc.sync.dma_start(
                bass.AP(u_dram.tensor, 0, [[8, 1], [1, 8]]), ex[:])
            u_all = spool.tile([128, 8], F32, tag="uall")
            g = nc.sync.dma_start(u_all[:],
                                  bass.AP(u_dram.tensor, 0, [[0, 128], [1, 8]]))
            _add_dep_helper(g.ins, udump.ins, reason="u RAW")

        # ================= phase D: Gauss-Jordan =================
        G = spool.tile([128, NST * 8 * 2 * 16], F32, tag="G")
        Gv = G[:].rearrange("p (s r k c) -> p s r k c", s=NST, r=8, k=2)
        for k in range(2):
            nc.vector.tensor_copy(Gv[:, :, :, k, 0:8], pav[:, :, 2 + k, :, :])
            nc.vector.tensor_copy(Gv[:, :, :, k, 8:16], pav[:, :, k, :, :])
        Gbase = G[:].ap[0][0]
        diagA = bass.AP(G.tensor, G.offset, [[Gbase, 128], [256, NST], [33, 8]])
        nc.vector.tensor_scalar_add(diagA, diagA, PAD_EPS)
        recs = spool.tile([128, NST * 8], F32, tag="recs")
        recsv = recs[:].rearrange("p (s r) -> p s r", s=NST)
        fv = spool.tile([128, NST * 8 * 2], F32, tag="fv")
        fvv = fv[:].rearrange("p (s r k) -> p s r k", s=NST, r=8)
        tt = spool.tile([128, NST * 8 * 16], F32, tag="tt")
        ttv = tt[:].rearrange("p (s r c) -> p s r c", s=NST, r=8)
        for k in range(8):
            piv = bass.AP(G.tensor, G.offset + k * 32 + k,
                          [[Gbase, 128], [256, NST], [0, 1]])
            nc.vector.reciprocal(recsv[:, :, k:k + 1], piv)
            colk = Gv[:, :, :, :, k]
            rb = recsv[:, :, k:k + 1].unsqueeze(2).broadcast_to([128, NST, 8, 2])
            nc.vector.tensor_mul(fvv, colk, rb)
            nc.scalar.mul(fvv[:, :, k, :], fvv[:, :, k, :], 0.0)
            ncol = 16 - k
            sh = [128, NST, 8, ncol]
            grow_r = Gv[:, :, k, 0, k:16].unsqueeze(2).broadcast_to(sh)
            grow_i = Gv[:, :, k, 1, k:16].unsqueeze(2).broadcast_to(sh)
            fr = fvv[:, :, :, 0].unsqueeze(3).broadcast_to(sh)
            fi = fvv[:, :, :, 1].unsqueeze(3).broadcast_to(sh)
            tv = ttv[:, :, :, 0:ncol]
            Gr = Gv[:, :, :, 0, k:16]
            Gi = Gv[:, :, :, 1, k:16]
            nc.vector.tensor_mul(tv, fr, grow_r)
            nc.vector.tensor_sub(Gr, Gr, tv)
            nc.vector.tensor_mul(tv, fi, grow_i)
            nc.vector.tensor_add(Gr, Gr, tv)
            nc.vector.tensor_mul(tv, fr, grow_i)
            nc.vector.tensor_sub(Gi, Gi, tv)
            nc.vector.tensor_mul(tv, fi, grow_r)
            nc.vector.tensor_sub(Gi, Gi, tv)
        Xs = spool.tile([128, NST * 8 * 2 * 8], F32, tag="Xs")
        Xm = Xs[:].rearrange("p (sr k c) -> p sr k c", k=2, c=8)
        Gm = G[:].rearrange("p (sr k c) -> p sr k c", k=2, c=16)
        rb2 = recs[:].unsqueeze(2).unsqueeze(3).broadcast_to([128, NST * 8, 2, 8])
        nc.vector.tensor_mul(Xm, Gm[:, :, :, 8:16], rb2)

        # ================= phase E: trace, w, wbd =================
        Xbase = Xs[:].ap[0][0]
        trv = spool.tile([128, NST * 2], F32, tag="trv")
        trvv = trv[:].rearrange("p (s k) -> p s k", s=NST)
        diagX = bass.AP(Xs.tensor, Xs.offset,
                        [[Xbase, 128], [128, NST], [8, 2], [17, 8]])
        nc.vector.tensor_reduce(trvv, diagX, axis=AX.X, op=OP.add)
        nc.vector.tensor_scalar_add(trvv[:, :, 0:1], trvv[:, :, 0:1], EPS_MVDR)
        ub = u_all[:].unsqueeze(1).broadcast_to([128, NST * 8 * 2, 8])
        wtmp = spool.tile([128, NST * 8 * 2 * 8], F32, tag="wtmp")
        wtm = wtmp[:].rearrange("p (srk c) -> p srk c", c=8)
        Xm2 = Xs[:].rearrange("p (srk c) -> p srk c", c=8)
        nc.vector.tensor_mul(wtm, Xm2, ub)
        wraw = spool.tile([128, NST * 8 * 2], F32, tag="wraw")
        wrv = wraw[:].rearrange("p (s r k) -> p s r k", s=NST, r=8)
        nc.vector.tensor_reduce(wraw[:].unsqueeze(2), wtm, axis=AX.X, op=OP.add)
        t2m = spool.tile([128, NST], F32, tag="t2m")
        t2mv = t2m[:].unsqueeze(2)
        nc.vector.tensor_mul(t2mv, trvv[:, :, 0:1], trvv[:, :, 0:1])
        tmi = spool.tile([128, NST], F32, tag="tmi")
        tmiv = tmi[:].unsqueeze(2)
        nc.vector.tensor_mul(tmiv, trvv[:, :, 1:2], trvv[:, :, 1:2])
        nc.vector.tensor_add(t2m[:], t2m[:], tmi[:])
        nc.vector.reciprocal(t2m[:], t2m[:])
        ctr = spool.tile([128, NST * 2], F32, tag="ctr")
        ctrv = ctr[:].rearrange("p (s k) -> p s k", s=NST)
        t2b = t2m[:].unsqueeze(2).broadcast_to([128, NST, 2])
        nc.vector.tensor_mul(ctrv, trvv, t2b)
        nc.scalar.mul(ctrv[:, :, 1:2], ctrv[:, :, 1:2], -1.0)
        wf = spool.tile([128, NST * 8 * 2], F32, tag="wf")
        wfv = wf[:].rearrange("p (s r k) -> p s r k", s=NST, r=8)
        cr = ctrv[:, :, 0:1].unsqueeze(2).broadcast_to([128, NST, 8, 1])
        ci_ = ctrv[:, :, 1:2].unsqueeze(2).broadcast_to([128, NST, 8, 1])
        wr_tmp = spool.tile([128, NST * 8], F32, tag="wrt")
        wrtv = wr_tmp[:].rearrange("p (s r) -> p s r", s=NST).unsqueeze(3)
        nc.vector.tensor_mul(wfv[:, :, :, 0:1], wrv[:, :, :, 0:1], cr)
        nc.vector.tensor_mul(wrtv, wrv[:, :, :, 1:2], ci_)
        nc.vector.tensor_sub(wfv[:, :, :, 0:1], wfv[:, :, :, 0:1], wrtv)
        nc.vector.tensor_mul(wfv[:, :, :, 1:2], wrv[:, :, :, 1:2], cr)
        nc.vector.tensor_mul(wrtv, wrv[:, :, :, 0:1], ci_)
        nc.vector.tensor_add(wfv[:, :, :, 1:2], wfv[:, :, :, 1:2], wrtv)
        # w12: v0 = (wr, -wi) pairs x_r; v1 = (wi, wr) pairs x_i  (conj(w))
        w12 = spool.tile([128, NST * 32], BF16, tag="w12")
        w12v = w12[:].rearrange("p (s v c o) -> p s v c o", s=NST, v=2, c=8)
        nc.vector.tensor_copy(w12v[:, :, 0, :, 0:1], wfv[:, :, :, 0:1])
        nc.scalar.mul(w12v[:, :, 0, :, 1:2], wfv[:, :, :, 1:2], -1.0)
        nc.vector.tensor_copy(w12v[:, :, 1, :, 0:1], wfv[:, :, :, 1:2])
        nc.vector.tensor_copy(w12v[:, :, 1, :, 1:2], wfv[:, :, :, 0:1])
        wdumps = []
        for st in range(NST):
            d = nc.sync.dma_start(
                bass.AP(w12_dram.tensor, st * 128 * 32, [[32, 128], [1, 32]]),
                w12[:, st * 32:(st + 1) * 32])
            wdumps.append(d)
        wbd = spool.tile([128, NFB * 64], BF16, tag="wbd")
        nc.scalar.dma_start(wbd[:].rearrange("p (b e) -> p b e", b=NFB),
                            bass.AP(wzero.tensor, 0, [[0, 128], [0, NFB], [1, 64]]))
        wpitch = wbd[:].ap[0][0]
        for ft in range(16):
            for vv_ in range(2):
                src = bass.AP(w12_dram.tensor, ft * 32 + vv_ * 16,
                              [[2, 8], [512, NFB], [1, 2]])
                dst = bass.AP(wbd.tensor,
                              wbd.offset + (8 * ft) * wpitch + vv_ * 32 + ft * 2,
                              [[wpitch, 8], [64, NFB], [1, 2]])
                geng = nc.scalar if (ft % 2 == 0) else nc.sync
                g = geng.dma_start(dst, src)
                for d in wdumps:
                    _add_dep_helper(g.ins, d.ins, reason="w12 RAW")

        # ================= phase F: beamform =================
        with ExitStack() as ctx:
            outp = ctx.enter_context(tc.tile_pool(name="outp", bufs=2))
            xfs = ctx.enter_context(tc.tile_pool(name="xfs", bufs=2))
            ppb = ctx.enter_context(tc.tile_pool(name="ppb", bufs=2, space="PSUM"))
            next_load = RES_FB

            def stream_to(limit):
                nonlocal next_load
                while next_load < min(NFB, limit):
                    fb2 = next_load
                    xr_t = xfs.tile([128, T], BF16, tag=f"xsr{fb2 % 4}",
                                    name=f"xsr{fb2}")
                    xi_t = xfs.tile([128, T], BF16, tag=f"xsi{fb2 % 4}",
                                    name=f"xsi{fb2}")
                    nc.gpsimd.dma_start(xr_t[:],
                                        xfc_r[fb2 * 128:(fb2 + 1) * 128, :])
                    nc.sync.dma_start(xi_t[:],
                                      xfc_i[fb2 * 128:(fb2 + 1) * 128, :])
                    xfr[fb2], xfi[fb2] = xr_t, xi_t
                    next_load += 1

            stream_to(RES_FB + 4)
            for gi in range(9):
                nfb_g = 4 if gi < 8 else 1
                stream_to((gi + 2) * 4)
                for (t0, tn) in [(0, 512), (512, 288)]:
                    ob = outp.tile([128, 512], F32, tag="ob", name=f"ob{gi}")
                    for q in range(nfb_g):
                        fb = gi * 4 + q
                        bp = ppb.tile([32, 512], F32, tag=f"bps{q % 2}",
                                      name=f"bps{q}")
                        nc.tensor.matmul(bp[0:32, 0:tn],
                                         wbd[:, fb * 64:fb * 64 + 32],
                                         xfr[fb][:, t0:t0 + tn],
                                         start=True, stop=False)
                        nc.tensor.matmul(bp[0:32, 0:tn],
                                         wbd[:, fb * 64 + 32:fb * 64 + 64],
                                         xfi[fb][:, t0:t0 + tn],
                                         start=False, stop=True)
                        eng = nc.vector.tensor_copy if q % 2 == 0 else nc.scalar.copy
                        eng(ob[32 * q:32 * q + 32, 0:tn], bp[0:32, 0:tn])
                    nc.scalar.dma_start(
                        out_dev[gi * 128:gi * 128 + 32 * nfb_g, t0:t0 + tn],
                        ob[0:32 * nfb_g, 0:tn])

    nc.compile()
    return nc


_NC_CACHE = None


def _get_nc():
    global _NC_CACHE
    if _NC_CACHE is None:
        _NC_CACHE = build_nc()
    return _NC_CACHE


def make_core_inputs(data_real, data_imag, mask_speech, mask_noise,
                     mlp_w, mlp_b, gvec_w, gvec_b, b):
    bf16 = ml_dtypes.bfloat16
    f8 = ml_dtypes.float8_e4m3
    xr = np.zeros((T, C, FP), np.float32)
    xi = np.zeros((T, C, FP), np.float32)
    xr[:, :, :F] = data_real[b]
    xi[:, :, :F] = data_imag[b]
    ms = np.zeros((T, C, FP), np.float32)
    mn = np.zeros((T, C, FP), np.float32)
    ms[:, :, :F] = np.transpose(np.asarray(mask_speech[b]), (2, 1, 0))
    mn[:, :, :F] = np.transpose(np.asarray(mask_noise[b]), (2, 1, 0))
    mw = np.zeros((NST * 128, A), np.float32)
    mw[:F] = mlp_w

    def thc(a):  # (T, C, FP) -> (T, NTH*FTH*C) with th-outer, f, c-inner
        return np.ascontiguousarray(
            a.reshape(T, C, NTH, FTH).transpose(0, 2, 3, 1).reshape(T, C * FP))

    return dict(
        xtc_r=thc(xr).astype(bf16),
        xtc_i=thc(xi).astype(bf16),
        xfc_r=np.ascontiguousarray(
            xr.transpose(2, 1, 0).reshape(NFB * 128, T)).astype(bf16),
        xfc_i=np.ascontiguousarray(
            xi.transpose(2, 1, 0).reshape(NFB * 128, T)).astype(bf16),
        mask_s=thc(ms).astype(bf16),
        mask_n=thc(mn).astype(bf16),
        mlp_w=mw.astype(bf16),
        mlp_b=np.asarray(mlp_b).reshape(1, A).astype(np.float32),
        gvec_w=np.asarray(gvec_w).reshape(1, A).astype(np.float32),
        gvec_b=np.asarray(gvec_b).reshape(1, 1).astype(np.float32),
        ones_bf=np.ones((128, 1), bf16),
        wzero=np.zeros((1, 64), bf16),
    )


def assemble_output(core_outs):
    out = np.zeros((B, T, F, 2), np.float32)
    for b in range(B):
        od = np.asarray(core_outs[b]["out_dev"]).reshape(9 * 128, T)
        # row = gi*128 + q*32 + fprime*2 + oc ; f = 16*(4*gi+q) + fprime
        rows = od.reshape(9, 4, 16, 2, T)
        full = rows.reshape(36, 16, 2, T).reshape(576, 2, T)
        out[b, :, :, 0] = full[:F, 0, :].T
        out[b, :, :, 1] = full[:F, 1, :].T
    return out


def kernel(**inputs):
    nc = _get_nc()
    in_maps = [
        make_core_inputs(inputs["data_real"], inputs["data_imag"],
                         inputs["mask_speech"], inputs["mask_noise"],
                         inputs["mlp_w"], inputs["mlp_b"],
                         inputs["gvec_w"], inputs["gvec_b"], b)
        for b in range(B)
    ]
    res = run_bass_kernel_spmd(nc, in_maps, list(range(B))).results
    return assemble_output(res)


if __name__ == "__main__":
    build_nc()
    print("built ok")
